# revision 6
# baseline (speedup 1.0000x reference)
"""MeanAggregator (GNN message passing) on 8 Trainium2 NeuronCores.

reference: out[i] = mean_j features[neigh_idx[i, j]]   (B=65536, S=25, D=128)

Strategy (data-parallel over batch):
  - Shard the batch 8 ways (8192 nodes per core); replicate the features
    table (each core gathers from its own HBM copy).
  - Per core: 64 node-tiles of 128 nodes. For each tile, 25 indirect DMA
    gathers (one per neighbor slot, 128 rows x 512B each, int32 row
    indices, one index per SBUF partition) fill a [128, 25, 128] f32 tile.
  - DVE reduces over the neighbor axis via a strided view, ACT scales by
    1/25, HWDGE stores the [128, 128] result tile.

Host side: reorder neigh_idx into the per-core [128, 1600] int32 layout
(idx_dev[p, t*25+j] = neigh_idx[c*8192 + t*128 + p, j]), run the SPMD
kernel on cores 0-7, concatenate the 8 output shards.
"""

import numpy as np

N_NODES = 500000
D = 128
BATCH = 65536
S = 25
N_CORES = 8
B_CORE = BATCH // N_CORES          # 8192
T = B_CORE // 128                  # 64 node-tiles per core

_cache = {}


def _split_excess_waits(nc, mybir):
    """Walrus codegen caps sync waits per instruction (1, or 2 for EVSEM).

    Tile's wait assigner can emit more; spill the excess onto freshly
    inserted NoOps on the same engine, placed right before the
    over-subscribed instruction.
    """
    n_spill = 0
    for f in nc.m.functions:
        for b in f.blocks:
            insts = list(b.instructions)
            out = []
            for ins in insts:
                si = ins.sync_info
                waits = list(si.on_wait) if si and si.on_wait else []
                cap = 2 if isinstance(ins, mybir.InstEventSemaphore) else 1
                if len(waits) > cap:
                    spill, keep = waits[:-cap], waits[-cap:]
                    for w in spill:
                        nop = mybir.InstNoOp(
                            name=f"I-waitspill-{n_spill}", ins=[], outs=[]
                        )
                        n_spill += 1
                        nop.engine = ins.engine
                        nop.sync_info = mybir.SyncInfo(on_wait=[w], on_update=[])
                        out.append(nop)
                    si.on_wait = keep
                out.append(ins)
            b.instructions = out
    return n_spill


def _build_program():
    from concourse import bass, mybir, tile

    nc = bass.Bass(target_bir_lowering=False, dynamic_dma_scratch_size=32768, num_swdge_queues=4)
    feat = nc.dram_tensor("features", [N_NODES, D], mybir.dt.float32,
                          kind="ExternalInput")
    idx = nc.dram_tensor("idx", [128, T * S], mybir.dt.int32,
                         kind="ExternalInput")
    out = nc.dram_tensor("out", [B_CORE, D], mybir.dt.float32,
                         kind="ExternalOutput")

    with tile.TileContext(nc) as tc:
        with tc.tile_pool(name="sbuf", bufs=6) as sbuf, \
             tc.tile_pool(name="small", bufs=6) as small, \
             tc.tile_pool(name="idxp", bufs=1) as idxp:
            # chunk the idx load so tile 0's gathers start ~2us earlier
            N_CHUNK = 4
            cw = T * S // N_CHUNK
            idx_tiles = []
            for ch in range(N_CHUNK):
                it = idxp.tile([128, cw], mybir.dt.int32, tag=f"idx{ch}")
                nc.sync.dma_start(out=it[:], in_=idx[:, ch * cw:(ch + 1) * cw])
                idx_tiles.append(it)
            for t in range(T):
                g = sbuf.tile([128, S, D], mybir.dt.float32, tag="g")
                for j in range(S):
                    col = t * S + j
                    it = idx_tiles[col // cw]
                    lcol = col % cw
                    nc.gpsimd.indirect_dma_start(
                        out=g[:, j, :],
                        out_offset=None,
                        in_=feat[:],
                        in_offset=bass.IndirectOffsetOnAxis(
                            ap=it[:, lcol:lcol + 1], axis=0,
                        ),
                    )
                r = small.tile([128, D], mybir.dt.float32, tag="r")
                # view [128, S, D] as [128, D, S]; reduce innermost (S)
                nc.vector.tensor_reduce(
                    out=r[:],
                    in_=g[:].rearrange("p j f -> p f j"),
                    axis=mybir.AxisListType.X,
                    op=mybir.AluOpType.add,
                )
                o = small.tile([128, D], mybir.dt.float32, tag="o")
                nc.scalar.mul(out=o[:], in_=r[:], mul=1.0 / S)
                nc.sync.dma_start(out=out[t * 128:(t + 1) * 128, :], in_=o[:])

    # spread indirect gathers across the 4 SWDGE queues
    qi = 0
    for f in nc.m.functions:
        for b in f.blocks:
            for ins in b.instructions:
                if isinstance(ins, mybir.InstDMACopy) and ins.queue == "qPoolDynamic":
                    ins.queue = f"qPoolDynamic{qi % 4 if qi % 4 else ''}"
                    ins.single_packet = True
                    qi += 1
    _split_excess_waits(nc, mybir)
    return nc


def _get_program():
    if "nc" not in _cache:
        _cache["nc"] = _build_program()
    return _cache["nc"]


def kernel(features: np.ndarray, neigh_idx: np.ndarray) -> np.ndarray:
    from concourse.bass_utils import run_bass_kernel_spmd

    features = np.ascontiguousarray(np.asarray(features), dtype=np.float32)
    neigh_idx = np.asarray(neigh_idx)
    assert features.shape == (N_NODES, D), features.shape
    assert neigh_idx.shape == (BATCH, S), neigh_idx.shape
    # per-core index layout: idx_dev[p, t*S+j] = neigh_idx[c*B + t*128 + p, j]
    idx32 = neigh_idx.astype(np.int32).reshape(N_CORES, T, 128, S)
    in_maps = []
    for c in range(N_CORES):
        idx_dev = np.ascontiguousarray(
            idx32[c].transpose(1, 0, 2).reshape(128, T * S)
        )
        in_maps.append({"features": features, "idx": idx_dev})

    nc = _get_program()
    res = run_bass_kernel_spmd(nc, in_maps, core_ids=list(range(N_CORES)))
    return np.concatenate([r["out"] for r in res.results], axis=0)
